# revision 10
# baseline (speedup 1.0000x reference)
"""BicausalNet Trainium2 kernel (8 NeuronCores, no cross-core communication).

Math reformulation (verified against the jax reference to 1e-5):
`_scramble_and_pad` is index-doubling mod M (M = 2L-1 = 8191) on the 8191
interior positions, and since 2^13 = 1 (mod 8191) the permutation bookkeeping
collapses.  With state u[i, p, c] on a circular axis i in Z_M:

  init: u[0:4096, 0] = embs;  u[4096:, 0] = mask;  u[:, 1] = mask
  layer k (k=0..11), offset o = 2^k:
    z[i,p] = u[i,p] @ Wc_k^T + b_k + u[(i+o)%M, 0] @ Wr_k^T + u[(i-o)%M, 0] @ Wl_k^T
    u'[i,p] = relu(z[i,p]) + u[i,p]
  output = (u12[0:4096, 0], u12[0:4096, 1])

Key structural facts used for sharding:
 - slot 0 evolves independently of slot 1 (stencil reads slot 0 only);
 - slot 1 at position m depends only on u0[m +- o] and u1[m]: slot-1
   positions never interact, and only positions [0, 4096) reach the output.

Sharding (8 cores, zero communication): core c owns batch c//2.  Every core
computes the full slot-0 circle (replicated inside the pair; it feeds all
stencil reads) plus one half of the output slot-1 range: core 2a owns slot-1
positions [0, 2048), core 2a+1 owns [2048, 4096).  The odd core stores its u0
circle rotated by 2048 (pure data prep on the host; the stencil is
rotation-invariant) so both cores run the same instruction stream with its
slot-1 columns aligned to u0 columns [0, 2048).

Circular wraparound: u0 is stored with a 511-column replicated tail margin
(cols M..M+510 mirror cols 0..510, maintained by one extra epilogue store per
layer), so every +-o stencil read is a single contiguous slice.

Compute dtype: bf16 operands, fp32 PSUM accumulation and epilogue (simulated
end-to-end rel err vs the fp32 reference: ~8e-3).
"""

import sys

for _p in ("/opt/trn_rl_repo", "/root/.axon_site/_ro/trn_rl_repo"):
    if _p not in sys.path:
        sys.path.insert(0, _p)

from contextlib import ExitStack

import numpy as np
import ml_dtypes

import concourse.bass as bass
import concourse.tile as tile
from concourse import bacc, mybir
from concourse.bass_utils import run_bass_kernel_spmd

B = 4
L = 4096
C = 384
M = 2 * L - 1          # 8191
NL = 12
P = 128
CC = C // P            # 3 channel chunks
NCORES = 8
NB = 512               # position block (one PSUM bank of fp32 output)
MARG = NB - 1          # wraparound margin
WU = M + MARG          # u0 buffer width
Q = 2048               # slot-1 positions per core
NBLK0 = (M + NB - 1) // NB   # 16 slot-0 blocks (last is 511 wide)
NBLK1 = Q // NB              # 4 slot-1 blocks

_cache = {}
import os as _os
REPS = int(_os.environ.get("KERNEL_REPS", "1"))  # timing aid: repeat the layer loop


def _build():
    nc = bacc.Bacc("TRN2", target_bir_lowering=False, debug=False,
                   num_devices=NCORES)
    bf16 = mybir.dt.bfloat16
    f32 = mybir.dt.float32

    u0i = nc.dram_tensor("u0i", [P, CC, M], bf16, kind="ExternalInput")
    u1i = nc.dram_tensor("u1i", [P, CC, Q], bf16, kind="ExternalInput")
    wt = nc.dram_tensor("wt", [NL, P, 3, CC, C], bf16, kind="ExternalInput")
    bi = nc.dram_tensor("bi", [P, NL, CC], f32, kind="ExternalInput")
    out0 = nc.dram_tensor("out0", [P, CC, L], bf16, kind="ExternalOutput")
    out1 = nc.dram_tensor("out1", [P, CC, Q], bf16, kind="ExternalOutput")

    with tile.TileContext(nc) as tc, ExitStack() as ctx:
        sb = ctx.enter_context(tc.tile_pool(name="sb", bufs=1))
        wpool = ctx.enter_context(tc.tile_pool(name="wp", bufs=2))
        stag = ctx.enter_context(tc.tile_pool(name="st", bufs=8))
        psum = ctx.enter_context(tc.tile_pool(name="ps", bufs=8, space="PSUM"))

        u0a = sb.tile([P, CC, WU], bf16, name="u0a")
        u0b = sb.tile([P, CC, WU], bf16, name="u0b")
        u1a = sb.tile([P, CC, Q], bf16, name="u1a")
        u1b = sb.tile([P, CC, Q], bf16, name="u1b")
        bias_sb = sb.tile([P, NL, CC], f32, name="bias_sb")

        nc.sync.dma_start(out=u0a[:, :, 0:M], in_=u0i.ap())
        nc.sync.dma_start(out=u0a[:, :, M:WU], in_=u0i.ap()[:, :, 0:MARG])
        nc.sync.dma_start(out=u1a[:, :, :], in_=u1i.ap())
        nc.sync.dma_start(out=bias_sb, in_=bi.ap())

        relu = mybir.ActivationFunctionType.Relu

        for k_rep in range(NL * REPS):
            k = k_rep % NL
            o = 1 << k
            u0, u1 = (u0a, u1a) if k % 2 == 0 else (u0b, u1b)
            u0n, u1n = (u0b, u1b) if k % 2 == 0 else (u0a, u1a)

            wsb = wpool.tile([P, 3, CC, C], bf16, tag="w")
            nc.sync.dma_start(out=wsb, in_=wt.ap()[k])

            def block(slot, a, n):
                # moving slices for (center, +o, -o); all single contiguous
                # reads thanks to the replicated tail margin
                sp = (a + o) % M
                sm = (a - o) % M
                for j in range(CC):
                    z = psum.tile([P, NB], mybir.dt.float32, tag="z")
                    for cc in range(CC):
                        if slot == 0:
                            mv_c = u0[:, cc, a:a + n]
                        else:
                            mv_c = u1[:, cc, a:a + n]
                        movs = (mv_c,
                                u0[:, cc, sp:sp + n],
                                u0[:, cc, sm:sm + n])
                        for mi in range(3):
                            nc.tensor.matmul(
                                z[:, 0:n],
                                wsb[:, mi, cc, j * P:(j + 1) * P],
                                movs[mi],
                                start=(cc == 0 and mi == 0),
                                stop=(cc == CC - 1 and mi == 2),
                            )
                    t = stag.tile([P, NB], mybir.dt.float32, tag="t")
                    nc.scalar.activation(t[:, 0:n], z[:, 0:n], relu,
                                         bias=bias_sb[:, k, j:j + 1])
                    if slot == 0:
                        nc.vector.tensor_add(u0n[:, j, a:a + n],
                                             t[:, 0:n], u0[:, j, a:a + n])
                        if a == 0:
                            # maintain the replicated wraparound tail
                            nc.vector.tensor_add(u0n[:, j, M:WU],
                                                 t[:, 0:MARG],
                                                 u0[:, j, 0:MARG])
                    else:
                        nc.vector.tensor_add(u1n[:, j, a:a + n],
                                             t[:, 0:n], u1[:, j, a:a + n])

            # last layer: only slot-0 positions [0, L) reach the output
            nblk0 = NBLK0 if k < NL - 1 else L // NB
            for blk in range(nblk0):
                a = blk * NB
                block(0, a, min(NB, M - a))
            for blk in range(NBLK1):
                block(1, blk * NB, NB)

        uf0, uf1 = (u0a, u1a) if NL % 2 == 0 else (u0b, u1b)
        nc.sync.dma_start(out=out0.ap(), in_=uf0[:, :, 0:L])
        nc.sync.dma_start(out=out1.ap(), in_=uf1[:, :, :])

    nc.compile()
    return nc


def _to_tile(x_cm):
    # [C, W] channel-major -> [P, CC, W]
    w = x_cm.shape[1]
    return np.ascontiguousarray(x_cm.reshape(CC, P, w).transpose(1, 0, 2))


def _prep_inputs(embs, mask_vals, w_left, w_center, w_right, bias):
    bf = ml_dtypes.bfloat16
    # wT[k, p, mi, cc, d] = W_mi[k][d, cc*128+p]  (mi: 0=center, 1=right, 2=left)
    wt = np.empty((NL, P, 3, CC, C), dtype=np.float32)
    for mi, w in enumerate((w_center, w_right, w_left)):
        t = np.ascontiguousarray(np.transpose(w, (0, 2, 1))).reshape(NL, CC, P, C)
        wt[:, :, mi, :, :] = np.transpose(t, (0, 2, 1, 3))
    wt = wt.astype(bf)
    bi = np.ascontiguousarray(
        np.transpose(bias.reshape(NL, CC, P), (2, 0, 1))).astype(np.float32)

    in_maps = []
    for core in range(NCORES):
        b = core // 2
        rot = (core % 2) * Q
        idx = (rot + np.arange(M)) % M
        u0 = np.where((idx < L)[None, :],
                      embs[b].T[:, np.clip(idx, 0, L - 1)],
                      mask_vals[b][:, None]).astype(np.float32)
        u1 = np.broadcast_to(mask_vals[b][:, None], (C, Q)).astype(np.float32)
        in_maps.append({
            "u0i": _to_tile(u0).astype(bf),
            "u1i": _to_tile(u1).astype(bf),
            "wt": wt,
            "bi": bi,
        })
    return in_maps


def kernel(embs, mask_vals, w_left, w_center, w_right, bias):
    embs = np.asarray(embs, dtype=np.float32)
    mask_vals = np.asarray(mask_vals, dtype=np.float32)
    w_left = np.asarray(w_left, dtype=np.float32)
    w_center = np.asarray(w_center, dtype=np.float32)
    w_right = np.asarray(w_right, dtype=np.float32)
    bias = np.asarray(bias, dtype=np.float32)

    if "nc" not in _cache:
        _cache["nc"] = _build()
    nc = _cache["nc"]

    in_maps = _prep_inputs(embs, mask_vals, w_left, w_center, w_right, bias)
    res = run_bass_kernel_spmd(nc, in_maps, core_ids=list(range(NCORES)))
    _cache["last_res"] = res

    def from_tile(t):  # [P, CC, W] -> [W, C]
        return t.astype(np.float32).transpose(1, 0, 2).reshape(C, -1).T

    o0 = np.empty((B, L, C), dtype=np.float32)
    o1 = np.empty((B, L, C), dtype=np.float32)
    for b in range(B):
        o0[b] = from_tile(res.results[2 * b]["out0"])
        o1[b, 0:Q] = from_tile(res.results[2 * b]["out1"])
        o1[b, Q:L] = from_tile(res.results[2 * b + 1]["out1"])
    return o0, o1


if __name__ == "__main__":
    rng = np.random.default_rng(0)
    ins = {
        "embs": rng.standard_normal((B, L, C), dtype=np.float32),
        "mask_vals": rng.standard_normal((B, C), dtype=np.float32),
        "w_left": rng.standard_normal((NL, C, C), dtype=np.float32) * 0.03,
        "w_center": rng.standard_normal((NL, C, C), dtype=np.float32) * 0.03,
        "w_right": rng.standard_normal((NL, C, C), dtype=np.float32) * 0.03,
        "bias": rng.standard_normal((NL, C), dtype=np.float32) * 0.03,
    }
    o0, o1 = kernel(**ins)
    print("ok", o0.shape, o1.shape, float(np.abs(o0).max()))


# revision 13
# speedup vs baseline: 1.9134x; 1.9134x over previous
"""BicausalNet Trainium2 kernel (8 NeuronCores, no cross-core communication).

Math reformulation (verified against the jax reference to 1e-5):
`_scramble_and_pad` is index-doubling mod M (M = 2L-1 = 8191) on the 8191
interior positions, and since 2^13 = 1 (mod 8191) the permutation bookkeeping
collapses.  With state u[i, p, c] on a circular axis i in Z_M:

  init: u[0:4096, 0] = embs;  u[4096:, 0] = mask;  u[:, 1] = mask
  layer k (k=0..11), offset o = 2^k:
    z[i,p] = u[i,p] @ Wc_k^T + b_k + u[(i+o)%M, 0] @ Wr_k^T + u[(i-o)%M, 0] @ Wl_k^T
    u'[i,p] = relu(z[i,p]) + u[i,p]
  output = (u12[0:4096, 0], u12[0:4096, 1])

Key structural facts used for sharding:
 - slot 0 evolves independently of slot 1 (stencil reads slot 0 only);
 - slot 1 at position m depends only on u0[m +- o] and u1[m]: slot-1
   positions never interact, and only positions [0, 4096) reach the output.

Sharding (8 cores, zero communication): core c owns batch c//2.  Every core
computes the full slot-0 circle (replicated inside the pair; it feeds all
stencil reads) plus one half of the output slot-1 range: core 2a owns slot-1
positions [0, 2048), core 2a+1 owns [2048, 4096).  The odd core stores its u0
circle rotated by 2048 (pure data prep on the host; the stencil is
rotation-invariant) so both cores run the same instruction stream with its
slot-1 columns aligned to u0 columns [0, 2048).

Circular wraparound: u0 is stored with a 511-column replicated tail margin
(cols M..M+510 mirror cols 0..510, maintained by one extra epilogue store per
layer), so every +-o stencil read is a single contiguous slice.

Compute dtype: bf16 operands, fp32 PSUM accumulation and epilogue (simulated
end-to-end rel err vs the fp32 reference: ~8e-3).
"""

import sys

for _p in ("/opt/trn_rl_repo", "/root/.axon_site/_ro/trn_rl_repo"):
    if _p not in sys.path:
        sys.path.insert(0, _p)

from contextlib import ExitStack

import numpy as np
import ml_dtypes

import concourse.bass as bass
import concourse.tile as tile
from concourse import bacc, mybir
from concourse.bass_utils import run_bass_kernel_spmd

B = 4
L = 4096
C = 384
M = 2 * L - 1          # 8191
NL = 12
P = 128
CC = C // P            # 3 channel chunks
NCORES = 8
NB = 512               # position block (one PSUM bank of fp32 output)
MARG = NB - 1          # wraparound margin
WU = M + MARG          # u0 buffer width
Q = 2048               # slot-1 positions per core
NBLK0 = (M + NB - 1) // NB   # 16 slot-0 blocks (last is 511 wide)
NBLK1 = Q // NB              # 4 slot-1 blocks

_cache = {}
import os as _os
REPS = int(_os.environ.get("KERNEL_REPS", "1"))  # timing aid: repeat the layer loop


def _build():
    nc = bacc.Bacc("TRN2", target_bir_lowering=False, debug=False,
                   num_devices=NCORES)
    bf16 = mybir.dt.bfloat16
    f32 = mybir.dt.float32

    u0i = nc.dram_tensor("u0i", [P, CC, M], bf16, kind="ExternalInput")
    u1i = nc.dram_tensor("u1i", [P, CC, Q], bf16, kind="ExternalInput")
    wt = nc.dram_tensor("wt", [NL, P, 3, CC, C], bf16, kind="ExternalInput")
    bi = nc.dram_tensor("bi", [P, NL, CC], f32, kind="ExternalInput")
    out0 = nc.dram_tensor("out0", [P, CC, L], bf16, kind="ExternalOutput")
    out1 = nc.dram_tensor("out1", [P, CC, Q], bf16, kind="ExternalOutput")

    with tile.TileContext(nc) as tc, ExitStack() as ctx:
        sb = ctx.enter_context(tc.tile_pool(name="sb", bufs=1))
        wpool = ctx.enter_context(tc.tile_pool(name="wp", bufs=2))
        stag = ctx.enter_context(tc.tile_pool(name="st", bufs=3))
        psum = ctx.enter_context(tc.tile_pool(name="ps", bufs=2, space="PSUM"))

        u0a = sb.tile([P, CC, WU], bf16, name="u0a")
        u0b = sb.tile([P, CC, WU], bf16, name="u0b")
        u1a = sb.tile([P, CC, Q], bf16, name="u1a")
        u1b = sb.tile([P, CC, Q], bf16, name="u1b")
        bias_sb = sb.tile([P, NL, CC], f32, name="bias_sb")

        nc.sync.dma_start(out=u0a[:, :, 0:M], in_=u0i.ap())
        nc.sync.dma_start(out=u0a[:, :, M:WU], in_=u0i.ap()[:, :, 0:MARG])
        nc.sync.dma_start(out=u1a[:, :, :], in_=u1i.ap())
        nc.sync.dma_start(out=bias_sb, in_=bi.ap())

        relu = mybir.ActivationFunctionType.Relu

        for k_rep in range(NL * REPS):
            k = k_rep % NL
            o = 1 << k
            u0, u1 = (u0a, u1a) if k % 2 == 0 else (u0b, u1b)
            u0n, u1n = (u0b, u1b) if k % 2 == 0 else (u0a, u1a)

            wsb = wpool.tile([P, 3, CC, C], bf16, tag="w")
            nc.sync.dma_start(out=wsb, in_=wt.ap()[k])

            def block(a, n, with_slot1):
                # moving slices for (center, +o, -o); all single contiguous
                # reads thanks to the replicated tail margin.  z is one
                # 3-bank PSUM tile [P, 3*NB]: column range j*NB.. holds
                # output-channel chunk j (each matmul output stays inside
                # one bank).  When with_slot1, the slot-1 block at the same
                # position is interleaved so each stationary weight load is
                # shared by two matmuls (and the +-o moving slices are
                # identical for both slots).
                sp = (a + o) % M
                sm = (a - o) % M
                z0 = psum.tile([P, CC * NB], mybir.dt.float32, tag="z")
                if with_slot1:
                    z1 = psum.tile([P, CC * NB], mybir.dt.float32, tag="z")
                for cc in range(CC):
                    movs = (u0[:, cc, a:a + n],
                            u0[:, cc, sp:sp + n],
                            u0[:, cc, sm:sm + n])
                    for mi in range(3):
                        st = (cc == 0 and mi == 0)
                        sp_ = (cc == CC - 1 and mi == 2)
                        for j in range(CC):
                            w_ap = wsb[:, mi, cc, j * P:(j + 1) * P]
                            nc.tensor.matmul(
                                z0[:, j * NB:j * NB + n], w_ap, movs[mi],
                                start=st, stop=sp_)
                            if with_slot1:
                                mv = (u1[:, cc, a:a + n] if mi == 0
                                      else movs[mi])
                                nc.tensor.matmul(
                                    z1[:, j * NB:j * NB + n], w_ap, mv,
                                    start=st, stop=sp_)

                def epilogue(z, u, un, a, n):
                    t = stag.tile([P, CC * NB], mybir.dt.float32, tag="t")
                    for j in range(CC):
                        nc.scalar.activation(
                            t[:, j * NB:j * NB + n], z[:, j * NB:j * NB + n],
                            relu, bias=bias_sb[:, k, j:j + 1])
                    nc.vector.tensor_add(
                        un[:, :, a:a + n],
                        t.rearrange("p (c w) -> p c w", c=CC)[:, :, 0:n],
                        u[:, :, a:a + n])
                    return t

                t0 = epilogue(z0, u0, u0n, a, n)
                if a == 0:
                    # maintain the replicated wraparound tail
                    nc.vector.tensor_add(
                        u0n[:, :, M:WU],
                        t0.rearrange("p (c w) -> p c w", c=CC)[:, :, 0:MARG],
                        u0[:, :, 0:MARG])
                if with_slot1:
                    epilogue(z1, u1, u1n, a, n)

            # last layer: only slot-0 positions [0, L) reach the output
            nblk0 = NBLK0 if k < NL - 1 else L // NB
            for blk in range(nblk0):
                a = blk * NB
                block(a, min(NB, M - a), with_slot1=(a < Q))

        uf0, uf1 = (u0a, u1a) if NL % 2 == 0 else (u0b, u1b)
        nc.sync.dma_start(out=out0.ap(), in_=uf0[:, :, 0:L])
        nc.sync.dma_start(out=out1.ap(), in_=uf1[:, :, :])

    nc.compile()
    return nc


def _to_tile(x_cm):
    # [C, W] channel-major -> [P, CC, W]
    w = x_cm.shape[1]
    return np.ascontiguousarray(x_cm.reshape(CC, P, w).transpose(1, 0, 2))


def _prep_inputs(embs, mask_vals, w_left, w_center, w_right, bias):
    bf = ml_dtypes.bfloat16
    # wT[k, p, mi, cc, d] = W_mi[k][d, cc*128+p]  (mi: 0=center, 1=right, 2=left)
    wt = np.empty((NL, P, 3, CC, C), dtype=np.float32)
    for mi, w in enumerate((w_center, w_right, w_left)):
        t = np.ascontiguousarray(np.transpose(w, (0, 2, 1))).reshape(NL, CC, P, C)
        wt[:, :, mi, :, :] = np.transpose(t, (0, 2, 1, 3))
    wt = wt.astype(bf)
    bi = np.ascontiguousarray(
        np.transpose(bias.reshape(NL, CC, P), (2, 0, 1))).astype(np.float32)

    in_maps = []
    for core in range(NCORES):
        b = core // 2
        rot = (core % 2) * Q
        idx = (rot + np.arange(M)) % M
        u0 = np.where((idx < L)[None, :],
                      embs[b].T[:, np.clip(idx, 0, L - 1)],
                      mask_vals[b][:, None]).astype(np.float32)
        u1 = np.broadcast_to(mask_vals[b][:, None], (C, Q)).astype(np.float32)
        in_maps.append({
            "u0i": _to_tile(u0).astype(bf),
            "u1i": _to_tile(u1).astype(bf),
            "wt": wt,
            "bi": bi,
        })
    return in_maps


def kernel(embs, mask_vals, w_left, w_center, w_right, bias):
    embs = np.asarray(embs, dtype=np.float32)
    mask_vals = np.asarray(mask_vals, dtype=np.float32)
    w_left = np.asarray(w_left, dtype=np.float32)
    w_center = np.asarray(w_center, dtype=np.float32)
    w_right = np.asarray(w_right, dtype=np.float32)
    bias = np.asarray(bias, dtype=np.float32)

    if "nc" not in _cache:
        _cache["nc"] = _build()
    nc = _cache["nc"]

    in_maps = _prep_inputs(embs, mask_vals, w_left, w_center, w_right, bias)
    res = run_bass_kernel_spmd(nc, in_maps, core_ids=list(range(NCORES)))
    _cache["last_res"] = res

    def from_tile(t):  # [P, CC, W] -> [W, C]
        return t.astype(np.float32).transpose(1, 0, 2).reshape(C, -1).T

    o0 = np.empty((B, L, C), dtype=np.float32)
    o1 = np.empty((B, L, C), dtype=np.float32)
    for b in range(B):
        o0[b] = from_tile(res.results[2 * b]["out0"])
        o1[b, 0:Q] = from_tile(res.results[2 * b]["out1"])
        o1[b, Q:L] = from_tile(res.results[2 * b + 1]["out1"])
    return o0, o1


if __name__ == "__main__":
    rng = np.random.default_rng(0)
    ins = {
        "embs": rng.standard_normal((B, L, C), dtype=np.float32),
        "mask_vals": rng.standard_normal((B, C), dtype=np.float32),
        "w_left": rng.standard_normal((NL, C, C), dtype=np.float32) * 0.03,
        "w_center": rng.standard_normal((NL, C, C), dtype=np.float32) * 0.03,
        "w_right": rng.standard_normal((NL, C, C), dtype=np.float32) * 0.03,
        "bias": rng.standard_normal((NL, C), dtype=np.float32) * 0.03,
    }
    o0, o1 = kernel(**ins)
    print("ok", o0.shape, o1.shape, float(np.abs(o0).max()))
